# revision 6
# baseline (speedup 1.0000x reference)
"""Trainium2 Bass kernel for ConcatVolume (stereo cost-volume concat).

Reference semantics (B=1, F=32, H=128, W=256, D=48, bins = arange(48)):
  vol_lr[0, 0:F,  d, h, w] = fl[0,:,h,w]        if w >= d      else 0
  vol_lr[0, F:2F, d, h, w] = fr[0,:,h,w-d]      if w >= d      else 0
  vol_rl[0, 0:F,  d, h, w] = fl[0,:,h,w+d]      if w <  W-d    else 0
  vol_rl[0, F:2F, d, h, w] = fr[0,:,h,w]        if w <  W-d    else 0
Returns (vol_lr, vol_rl), each [1, 2F, D, H, W] f32 (~403 MB each).

Strategy: the whole problem is data movement (memory-bound). Shard the D
axis across the 8 cores (6 bins/core). To keep a single SPMD program with
compile-time access patterns, the host passes each core *windowed* views of
zero-padded inputs so that per-(local bin j) source offsets are static:

  flp  = (fl ++ 48 zero cols)[base : base+261]   -> rl-left  src col = w + j
  frp  = (48 zero cols ++ fr)[43-base : 304-base]-> lr-right src col = w + 5 - j
  fl48 = fl[:, :, 48:]  (mask w>=d always true there)    -> lr-left cols 48..255
  fr208= fr[:, :, :208] (mask w<W-d always true there)   -> rl-right cols 0..207
  p1[j] = fl[:, :, :48]  * (w >= d)   boundary strip, host-masked
  p2[j] = fr[:, :, 208:] * (w < W-d)  boundary strip, host-masked

Device work: stage the four reused tensors in SBUF once (~15 MB), then per
local bin j issue six DMA stores (4 big SBUF->DRAM shifted copies + 2 small
DRAM->DRAM boundary strips). Everything is DMA; no compute engines.
"""

import numpy as np

B, F, H, W, D = 1, 32, 128, 256, 48
NCORES = 8
DPC = D // NCORES  # 6 bins per core
PAD = 48  # > max disparity (47)
WIN = W + DPC - 1  # 261: window width covering all 6 shifts

_cache = {}


def _build_program():
    import concourse.bacc as bacc
    import concourse.mybir as mybir
    import concourse.tile as tile

    nc = bacc.Bacc(
        "TRN2",
        target_bir_lowering=False,
        debug=False,
        enable_asserts=False,
        num_devices=NCORES,
    )

    f32 = mybir.dt.float32
    # staging inputs come in SBUF-friendly layout [H, F*width] (host transposes)
    flp = nc.dram_tensor("flp", [H, F * WIN], f32, kind="ExternalInput").ap()
    frp = nc.dram_tensor("frp", [H, F * WIN], f32, kind="ExternalInput").ap()
    fl48 = nc.dram_tensor("fl48", [H, F * (W - PAD)], f32, kind="ExternalInput").ap()
    fr208 = nc.dram_tensor("fr208", [H, F * (W - PAD)], f32, kind="ExternalInput").ap()
    p1 = nc.dram_tensor("p1", [DPC, F, H, PAD], f32, kind="ExternalInput").ap()
    p2 = nc.dram_tensor("p2", [DPC, F, H, PAD], f32, kind="ExternalInput").ap()
    olr = nc.dram_tensor("olr", [2 * F, DPC, H, W], f32, kind="ExternalOutput").ap()
    orl = nc.dram_tensor("orl", [2 * F, DPC, H, W], f32, kind="ExternalOutput").ap()

    with tile.TileContext(nc) as tc:
        with tc.tile_pool(name="stage", bufs=1) as pool:
            # SBUF layout: partition = h (128), free = f*width + w
            s_flp = pool.tile([H, F * WIN], f32, tag="s_flp")
            s_frp = pool.tile([H, F * WIN], f32, tag="s_frp")
            s_fl48 = pool.tile([H, F * (W - PAD)], f32, tag="s_fl48")
            s_fr208 = pool.tile([H, F * (W - PAD)], f32, tag="s_fr208")

            nc.sync.dma_start(s_flp[:], flp)
            nc.scalar.dma_start(s_frp[:], frp)
            nc.sync.dma_start(s_fl48[:], fl48)
            nc.scalar.dma_start(s_fr208[:], fr208)

            # SBUF views with partition (h) outermost: [h, f, w]
            v_flp = s_flp[:].rearrange("h (f w) -> h f w", f=F)
            v_frp = s_frp[:].rearrange("h (f w) -> h f w", f=F)
            v_fl48 = s_fl48[:].rearrange("h (f w) -> h f w", f=F)
            v_fr208 = s_fr208[:].rearrange("h (f w) -> h f w", f=F)

            def hfw(dram_slab):
                # DRAM slab [f, h, w] -> AP enumerated [h, f, w] to match SBUF
                return dram_slab.transpose([1, 0, 2])

            for j in range(DPC):
                # lr-left: cols 48.. from fl48, cols 0..47 from host strip p1[j]
                nc.sync.dma_start(hfw(olr[0:F, j, :, PAD:W]), v_fl48)
                nc.sync.dma_start(olr[0:F, j, :, 0:PAD], p1[j])
                # lr-right: shifted window of padded fr
                nc.scalar.dma_start(
                    hfw(olr[F : 2 * F, j, :, :]),
                    v_frp[:, :, DPC - 1 - j : DPC - 1 - j + W],
                )
                # rl-left: shifted window of padded fl
                nc.sync.dma_start(hfw(orl[0:F, j, :, :]), v_flp[:, :, j : j + W])
                # rl-right: cols 0..207 from fr208, cols 208.. from strip p2[j]
                nc.scalar.dma_start(hfw(orl[F : 2 * F, j, :, 0 : W - PAD]), v_fr208)
                nc.scalar.dma_start(orl[F : 2 * F, j, :, W - PAD : W], p2[j])

    nc.compile()
    return nc


def _get_program():
    if "nc" not in _cache:
        _cache["nc"] = _build_program()
    return _cache["nc"]


def _host_prep(fl, fr):
    """Build the 8 per-core input maps. fl/fr: [F, H, W] f32 contiguous."""
    z = np.zeros((F, H, PAD), dtype=np.float32)
    fl_pad = np.concatenate([fl, z], axis=2)  # [F, H, 304]
    fr_pad = np.concatenate([z, fr], axis=2)  # [F, H, 304]
    fl48 = np.ascontiguousarray(fl[:, :, PAD:W])
    fr208 = np.ascontiguousarray(fr[:, :, 0 : W - PAD])

    def to_sbuf_layout(x):
        # [F, H, width] -> [H, F*width]
        Fv, Hv, Wv = x.shape
        return np.ascontiguousarray(np.transpose(x, (1, 0, 2)).reshape(Hv, Fv * Wv))

    fl48 = to_sbuf_layout(fl48)
    fr208 = to_sbuf_layout(fr208)

    w48 = np.arange(PAD)  # mask index for strips
    in_maps = []
    for c in range(NCORES):
        base = DPC * c
        flp = to_sbuf_layout(fl_pad[:, :, base : base + WIN])
        frp = to_sbuf_layout(fr_pad[:, :, 43 - base : 43 - base + WIN])
        ds = base + np.arange(DPC)  # [6]
        # p1[j,f,h,w] = fl[f,h,w] if w >= d_j else 0    (w in [0,48))
        m1 = (w48[None, :] >= ds[:, None])[:, None, None, :]  # [6,1,1,48]
        p1 = np.ascontiguousarray(
            np.where(m1, fl[None, :, :, 0:PAD], np.float32(0.0)), dtype=np.float32
        )
        # p2[j,f,h,k] = fr[f,h,208+k] if 208+k < W-d_j else 0
        m2 = ((W - PAD + w48)[None, :] < (W - ds)[:, None])[:, None, None, :]
        p2 = np.ascontiguousarray(
            np.where(m2, fr[None, :, :, W - PAD : W], np.float32(0.0)),
            dtype=np.float32,
        )
        in_maps.append(
            {
                "flp": flp,
                "frp": frp,
                "fl48": fl48,
                "fr208": fr208,
                "p1": p1,
                "p2": p2,
            }
        )
    return in_maps


def _get_exec():
    """Build (once) a persistent jitted SPMD executor for the bass program.

    Modeled on concourse.bass2jax.run_bass_via_pjrt, but cached so repeat
    calls don't re-trace/re-compile, and without output-buffer donation so
    the same callable can be invoked repeatedly (timing loops).
    """
    if "exec" in _cache:
        return _cache["exec"]

    import jax
    import concourse.mybir as mybir
    from jax.sharding import Mesh, PartitionSpec
    from jax.experimental.shard_map import shard_map
    from concourse.bass2jax import (
        _bass_exec_p,
        install_neuronx_cc_hook,
        partition_id_tensor,
    )

    nc = _get_program()
    install_neuronx_cc_hook()

    partition_name = (
        nc.partition_id_tensor.name if nc.partition_id_tensor else None
    )
    in_names, out_names, out_avals = [], [], []
    for alloc in nc.m.functions[0].allocations:
        if not isinstance(alloc, mybir.MemoryLocationSet):
            continue
        name = alloc.memorylocations[0].name
        if alloc.kind == "ExternalInput":
            if name != partition_name:
                in_names.append(name)
        elif alloc.kind == "ExternalOutput":
            out_names.append(name)
            out_avals.append(
                jax.core.ShapedArray(
                    tuple(alloc.tensor_shape), mybir.dt.np(alloc.dtype)
                )
            )
    n_params = len(in_names)
    all_names = in_names + out_names
    if partition_name is not None:
        all_names = all_names + [partition_name]

    def _body(*args):
        operands = list(args)
        if partition_name is not None:
            operands.append(partition_id_tensor())
        outs = _bass_exec_p.bind(
            *operands,
            out_avals=tuple(out_avals),
            in_names=tuple(all_names),
            out_names=tuple(out_names),
            lowering_input_output_aliases=(),
            sim_require_finite=True,
            sim_require_nnan=True,
            nc=nc,
        )
        return tuple(outs)

    devices = jax.devices()[:NCORES]
    mesh = Mesh(np.asarray(devices), ("core",))
    nin = n_params + len(out_names)
    sharded = jax.jit(
        shard_map(
            _body,
            mesh=mesh,
            in_specs=(PartitionSpec("core"),) * nin,
            out_specs=(PartitionSpec("core"),) * len(out_names),
            check_rep=False,
        ),
        keep_unused=True,
    )
    zeros = [
        np.zeros((NCORES * a.shape[0], *a.shape[1:]), a.dtype) for a in out_avals
    ]
    _cache["exec"] = (sharded, in_names, out_names, out_avals, zeros)
    return _cache["exec"]


def _run(features_left, features_right, bins, trace=False):
    fl = np.ascontiguousarray(np.asarray(features_left, dtype=np.float32)[0])
    fr = np.ascontiguousarray(np.asarray(features_right, dtype=np.float32)[0])
    in_maps = _host_prep(fl, fr)
    sharded, in_names, out_names, out_avals, zeros = _get_exec()
    concat_in = [
        np.concatenate([in_maps[c][name] for c in range(NCORES)], axis=0)
        for name in in_names
    ]
    out_arrs = sharded(*concat_in, *zeros)
    outs = {
        name: np.asarray(out_arrs[i]).reshape(NCORES, *out_avals[i].shape)
        for i, name in enumerate(out_names)
    }
    vol_lr = np.empty((B, 2 * F, D, H, W), dtype=np.float32)
    vol_rl = np.empty((B, 2 * F, D, H, W), dtype=np.float32)
    for c in range(NCORES):
        vol_lr[0, :, DPC * c : DPC * (c + 1)] = outs["olr"][c]
        vol_rl[0, :, DPC * c : DPC * (c + 1)] = outs["orl"][c]
    return (vol_lr, vol_rl), None


def _reference_np(features_left, features_right, bins):
    """Numpy fallback for unexpected bins (kept for robustness)."""
    fl = np.asarray(features_left, dtype=np.float32)
    fr = np.asarray(features_right, dtype=np.float32)
    bins = np.asarray(bins)
    Bv, Fv, Hv, Wv = fl.shape
    w = np.arange(Wv)
    b = bins[:, None]
    idx_m = np.clip(w[None, :] - b, 0, Wv - 1)
    idx_p = np.clip(w[None, :] + b, 0, Wv - 1)
    m_lr = (w[None, :] >= b)[None, None, :, None, :]
    m_rl = (w[None, :] < Wv - b)[None, None, :, None, :]
    g_r = np.transpose(fr[:, :, :, idx_m], (0, 1, 3, 2, 4))
    g_l = np.transpose(fl[:, :, :, idx_p], (0, 1, 3, 2, 4))
    bl = fl[:, :, None, :, :]
    br = fr[:, :, None, :, :]
    zero = np.float32(0.0)
    vol_lr = np.concatenate(
        [np.where(m_lr, bl, zero), np.where(m_lr, g_r, zero)], axis=1
    )
    vol_rl = np.concatenate(
        [np.where(m_rl, g_l, zero), np.where(m_rl, br, zero)], axis=1
    )
    return vol_lr.astype(np.float32), vol_rl.astype(np.float32)


def kernel(features_left, features_right, bins):
    fl = np.asarray(features_left)
    fr = np.asarray(features_right)
    b = np.asarray(bins)
    if (
        fl.shape != (B, F, H, W)
        or fr.shape != (B, F, H, W)
        or b.shape != (D,)
        or not np.array_equal(b, np.arange(D))
    ):
        return _reference_np(features_left, features_right, bins)
    out, _ = _run(fl, fr, b, trace=False)
    return out
